# revision 7
# baseline (speedup 1.0000x reference)
"""Graph multi-head attention (GNN message passing) on 8 Trainium2 NeuronCores.

v2 strategy (dst-sharded edge parallelism, no collectives, dma_gather bulk
indirect loads):
  - Host: sort edges by dst, split nodes into 8 contiguous ranges with ~equal
    edge counts. Each core owns all incoming edges of its node range, so the
    per-dst segment softmax is core-local.
  - Each core builds the full projected K/V table (kv[n] = [k(n)||v(n)],
    128 f32 = 512B rows) in HBM via bf16 matmuls, plus a local q table.
  - Edges are packed into 4-slot virtual rows keyed by (dst node, src chunk),
    where chunk = src // 32768. Rows are tiled 128-per-tile (whole (node,chunk)
    groups, <=127 distinct nodes per tile), tiles grouped into 4 chunk regions
    padded to a uniform supertile count so the SPMD program is identical on
    every core. Per-edge K/V rows are fetched with ONE dma_gather custom
    instruction per supertile (int16 indices relative to the supertile's chunk
    base) instead of per-edge SWDGE descriptors.
  - Scores/softmax: exp without max-subtraction (scores are O(sigma=1), no
    overflow); per-row slot sums on DVE; rows -> node columns via one-hot
    matmul per tile; per-(node,region) partials land in a bf16 comb table;
    final phase dma_gathers the <=4 partials per node, sums, normalizes, and
    applies the output projection.
"""

import os
from contextlib import ExitStack

import numpy as np

N = 100000
E = 1600000
DIM = 64
H = 4
DK = DIM // H
NCORES = 8

D_PAD = 4            # edge slots per virtual row
TC = 8               # tiles per supertile
CHUNK = 32768        # kv_tab rows addressable per gather (int16 idx)
NCHUNK = 4
KV_PAD = ((N + 127) // 128) * 128
NODES_PAD = 12544    # max per-core node count, = 7 * 1792
SLAB = 1792          # F-phase nodes per slab (14 cols of 128)
NSLAB = NODES_PAD // SLAB
TRASH_COL = 127      # one-hot column reserved for dummy rows (always zero)


def _pack_idx16(pos_idx):
    """positions (len%16==0) -> [128, n/16] int16 (16-partition wrap, x8)."""
    n = len(pos_idx)
    blk = np.ascontiguousarray(np.asarray(pos_idx, np.int16).reshape(n // 16, 16).T)
    return np.tile(blk, (8, 1))


def _host_prep(src, dst):
    src = np.asarray(src).astype(np.int64)
    dst = np.asarray(dst).astype(np.int64)
    order = np.argsort(dst, kind="stable")
    ssrc = src[order].astype(np.int32)
    deg = np.bincount(dst, minlength=N).astype(np.int64)
    cum = np.concatenate([[0], np.cumsum(deg)])

    bounds = [0]
    for c in range(1, NCORES):
        t = round(c * E / NCORES)
        n = int(np.searchsorted(cum, t, side="left"))
        n = min(max(n, bounds[-1] + 1), N - (NCORES - c))
        bounds.append(n)
    bounds.append(N)

    cores = []
    for c in range(NCORES):
        n0, n1 = bounds[c], bounds[c + 1]
        nn = n1 - n0
        e0, e1 = cum[n0], cum[n1]
        es = ssrc[e0:e1]                       # src of this core's edges (dst-sorted)
        ed = (dst[order][e0:e1] - n0).astype(np.int64)  # local dst
        ch = (es // CHUNK).astype(np.int64)
        # group edges by (node, chunk): stable sort
        key = ed * NCHUNK + ch
        eord = np.argsort(key, kind="stable")
        gsrc = es[eord]
        gkey = key[eord]
        # per-(node,chunk) group counts
        cnt = np.bincount(gkey, minlength=nn * NCHUNK).reshape(nn, NCHUNK)
        gstart = np.concatenate([[0], np.cumsum(cnt.reshape(-1))])  # per key
        rows_nc = -(-cnt // D_PAD)  # ceil; 0 where cnt==0
        # tile packing per chunk region
        core_tiles = []            # list per chunk: list of tiles; tile = list of (node, nrows, d)
        for cc in range(NCHUNK):
            nodes = np.nonzero(cnt[:, cc])[0]
            tiles = []
            cur, rt = [], 0
            for nd in nodes:
                r = int(rows_nc[nd, cc])
                if rt + r > 128 or len(cur) + 1 > TRASH_COL:
                    tiles.append(cur)
                    cur, rt = [], 0
                cur.append(nd)
                rt += r
            if cur:
                tiles.append(cur)
            core_tiles.append(tiles)
        cores.append(dict(n0=n0, n1=n1, nn=nn, cnt=cnt, rows_nc=rows_nc,
                          gsrc=gsrc, gstart=gstart, tiles=core_tiles))

    # uniform per-region supertile counts across cores
    ST_C = [max(-(-len(c["tiles"][cc]) // TC) for c in cores) for cc in range(NCHUNK)]
    T_C = [s * TC for s in ST_C]          # tiles per region (uniform)
    ST = sum(ST_C)
    NT = sum(T_C)
    REG_TILE0 = np.concatenate([[0], np.cumsum(T_C)])  # global tile idx of region start
    REG_ROW0 = [int(REG_TILE0[cc]) * 128 for cc in range(NCHUNK)]  # comb2 row offset

    for c in cores:
        nn = c["nn"]
        kv_idx = np.zeros((NT, D_PAD, 128), np.int16)   # [tile, slot j, partition]
        q_idx = np.zeros((NT, 128), np.int16)
        msk = np.full((NT, 128, D_PAD, H), -1e30, np.float32)
        crw = np.full((NT, 128), TRASH_COL, np.int16)
        fidx = np.full((NCHUNK, NODES_PAD), 0, np.int16)

        for cc in range(NCHUNK):
            tiles = c["tiles"][cc]
            base = cc * CHUNK
            for tl, nodes in enumerate(tiles):
                gt = int(REG_TILE0[cc]) + tl
                p = 0
                for col, nd in enumerate(nodes):
                    d = int(c["cnt"][nd, cc])
                    r = int(c["rows_nc"][nd, cc])
                    g0 = int(c["gstart"][nd * NCHUNK + cc])
                    ee = c["gsrc"][g0:g0 + d] - base       # rel src ids
                    fidx[cc, nd] = col * T_C[cc] + tl
                    for k in range(r):
                        a, b = k * D_PAD, min((k + 1) * D_PAD, d)
                        kv_idx[gt, 0:b - a, p] = ee[a:b]
                        msk[gt, p, 0:b - a, :] = 0.0
                        q_idx[gt, p] = nd
                        crw[gt, p] = col
                        p += 1
            # dummy idx for nodes with no partial in this region -> zero row
            none = c["cnt"][:, cc] == 0
            fidx[cc, :nn][none[:nn] if len(none) >= nn else none] = TRASH_COL * T_C[cc]
            fidx[cc, nn:] = TRASH_COL * T_C[cc]

        # reshape to DMA layouts
        # kv positions per supertile: pos = (lt*D_PAD + j)*128 + p, lt in [0,8)
        kv_pos = kv_idx.reshape(ST, TC, D_PAD, 128).reshape(ST, TC * D_PAD * 128)
        c["kv_idx16"] = np.concatenate(
            [_pack_idx16(kv_pos[s]) for s in range(ST)], axis=1)  # [128, ST*256]
        q_pos = q_idx.reshape(ST, TC * 128)
        c["q_idx16"] = np.concatenate(
            [_pack_idx16(q_pos[s]) for s in range(ST)], axis=1)   # [128, ST*64]
        # msk layout per supertile: [128 part, (lt, j), h]
        m = msk.reshape(ST, TC, 128, D_PAD, H).transpose(0, 2, 1, 3, 4)
        c["msk"] = m.reshape(ST, 128, TC * D_PAD * H).transpose(1, 0, 2).reshape(
            128, ST * TC * D_PAD * H).copy()
        cr = crw.reshape(ST, TC, 128).transpose(2, 0, 1)
        c["crw16"] = cr.reshape(128, ST * TC).copy()
        c["fidx16"] = np.concatenate(
            [_pack_idx16(fidx[cc]) for cc in range(NCHUNK)], axis=1)  # [128, 4*784]
        del c["gsrc"], c["gstart"], c["tiles"], c["cnt"], c["rows_nc"]

    meta = dict(ST=ST, NT=NT, ST_C=ST_C, T_C=T_C, REG_ROW0=REG_ROW0,
                REG_TILE0=[int(x) for x in REG_TILE0])
    return cores, meta


def _build_program(meta):
    import concourse.bass as bass
    import concourse.tile as tile
    from concourse import bacc, mybir, library_config

    f32 = mybir.dt.float32
    bf16 = mybir.dt.bfloat16
    i16 = mybir.dt.int16

    ST, NT = meta["ST"], meta["NT"]
    ST_C, T_C = meta["ST_C"], meta["T_C"]
    REG_ROW0 = meta["REG_ROW0"]

    # region of each supertile (uniform across cores)
    st_region = []
    for cc in range(NCHUNK):
        st_region += [cc] * ST_C[cc]

    nc = bacc.Bacc("TRN2", target_bir_lowering=False, debug=False,
                   num_devices=NCORES)

    keyT = nc.dram_tensor("keyT", [DIM, KV_PAD], bf16, kind="ExternalInput").ap()
    valT = nc.dram_tensor("valT", [DIM, KV_PAD], bf16, kind="ExternalInput").ap()
    qT = nc.dram_tensor("qT", [DIM, NODES_PAD], bf16, kind="ExternalInput").ap()
    wkv = nc.dram_tensor("wkv", [128, 128], bf16, kind="ExternalInput").ap()
    bkv = nc.dram_tensor("bkv", [128, 128], f32, kind="ExternalInput").ap()
    wqT = nc.dram_tensor("wqT", [DIM, DIM], bf16, kind="ExternalInput").ap()
    bq = nc.dram_tensor("bq", [128, DIM], f32, kind="ExternalInput").ap()
    woT = nc.dram_tensor("woT", [DIM, DIM], f32, kind="ExternalInput").ap()
    bo = nc.dram_tensor("bo", [128, DIM], f32, kind="ExternalInput").ap()
    identT = nc.dram_tensor("identT", [128, 128], f32, kind="ExternalInput").ap()
    iotaT = nc.dram_tensor("iotaT", [128, 128], f32, kind="ExternalInput").ap()
    kv_idx16 = nc.dram_tensor("kv_idx16", [128, ST * TC * D_PAD * 8], i16,
                              kind="ExternalInput").ap()
    q_idx16 = nc.dram_tensor("q_idx16", [128, ST * TC * 8], i16,
                             kind="ExternalInput").ap()
    mskap = nc.dram_tensor("msk", [128, ST * TC * D_PAD * H], f32,
                           kind="ExternalInput").ap()
    crw16 = nc.dram_tensor("crw16", [128, ST * TC], i16, kind="ExternalInput").ap()
    fidx16 = nc.dram_tensor("fidx16", [128, NCHUNK * NODES_PAD // 16], i16,
                            kind="ExternalInput").ap()
    out = nc.dram_tensor("out", [NODES_PAD, DIM], f32, kind="ExternalOutput").ap()

    kv_tab = nc.dram_tensor("kv_tab", [KV_PAD, 128], f32, kind="Internal")
    q_tab = nc.dram_tensor("q_tab", [NODES_PAD, DIM], f32, kind="Internal")
    comb2 = nc.dram_tensor("comb2", [NT * 128, 128], bf16,
                           kind="ExternalOutput" if os.environ.get("KERNEL_DEBUG")
                           else "Internal")

    KT = KV_PAD // 128
    QT = NODES_PAD // 128
    CH = 8
    SCOLS = TC * D_PAD          # 32 gather columns per supertile
    FCOLS = SLAB // 128         # 14

    dbg = {}
    if os.environ.get("KERNEL_DEBUG"):
        for nm, shape in [("dbg_kvg", [128, SCOLS * 128]),
                          ("dbg_sco", [128, SCOLS * H]),
                          ("dbg_ad", [128, TC * 68]),
                          ("dbg_qg", [128, TC * DIM])]:
            dbg[nm] = nc.dram_tensor(nm, shape, f32, kind="ExternalOutput").ap()

    with tile.TileContext(nc) as tc, ExitStack() as ctx:
        consts = ctx.enter_context(tc.tile_pool(name="consts", bufs=1))
        ld = ctx.enter_context(tc.tile_pool(name="ld", bufs=2))
        kvp = ctx.enter_context(tc.tile_pool(name="kvp", bufs=3))
        gat = ctx.enter_context(tc.tile_pool(name="gat", bufs=2))
        work = ctx.enter_context(tc.tile_pool(name="work", bufs=2))
        smal = ctx.enter_context(tc.tile_pool(name="smal", bufs=2))
        outp = ctx.enter_context(tc.tile_pool(name="outp", bufs=3))
        ps = ctx.enter_context(tc.tile_pool(name="ps", bufs=3, space="PSUM"))

        nc.gpsimd.load_library(library_config.mlp)

        wkv_sb = consts.tile([128, 128], bf16)
        nc.sync.dma_start(wkv_sb[:], wkv[:, :])
        bkv_sb = consts.tile([128, 128], f32)
        nc.sync.dma_start(bkv_sb[:], bkv[:, :])
        wq_sb = consts.tile([DIM, DIM], bf16)
        nc.sync.dma_start(wq_sb[:], wqT[:, :])
        bq_sb = consts.tile([128, DIM], f32)
        nc.sync.dma_start(bq_sb[:], bq[:, :])
        wo_sb = consts.tile([DIM, DIM], f32)
        nc.sync.dma_start(wo_sb[:], woT[:, :])
        bo_sb = consts.tile([128, DIM], f32)
        nc.sync.dma_start(bo_sb[:], bo[:, :])
        ident = consts.tile([128, 128], f32)
        nc.sync.dma_start(ident[:], identT[:, :])
        iota_f = consts.tile([128, 128], f32)
        nc.sync.dma_start(iota_f[:], iotaT[:, :])

        def ap4(t, dims, extra_off=0):
            a = t[:]
            return bass.AP(a.tensor, a.offset + extra_off, [list(a.ap[0])] + dims)

        # ---- P1: full kv table ----
        for jc in range(0, KT, CH):
            w = min(CH, KT - jc)
            lhs8 = ld.tile([128, CH * 128], bf16, tag="kt")
            nc.sync.dma_start(lhs8[0:DIM, : w * 128],
                              keyT[:, jc * 128:(jc + w) * 128])
            nc.scalar.dma_start(lhs8[DIM:128, : w * 128],
                                valT[:, jc * 128:(jc + w) * 128])
            kv_sb8 = kvp.tile([128, CH * 128], f32, tag="kvout")
            for j in range(w):
                acc = ps.tile([128, 128], f32, space="PSUM", tag="mm")
                nc.tensor.matmul(out=acc[:],
                                 lhsT=lhs8[:, j * 128:(j + 1) * 128],
                                 rhs=wkv_sb[:], start=True, stop=True)
                nc.vector.tensor_tensor(
                    out=kv_sb8[:, j * 128:(j + 1) * 128], in0=acc[:],
                    in1=bkv_sb[:], op=mybir.AluOpType.add)
            nc.sync.dma_start(
                bass.AP(kv_tab.ap()[:, :].tensor, jc * 128 * 128,
                        [[128, 128], [128 * 128, w], [1, 128]]),
                bass.AP(kv_sb8[:].tensor, kv_sb8[:].offset,
                        [list(kv_sb8[:].ap[0]), [128, w], [1, 128]]))

        # ---- P2: local q table ----
        for jc in range(0, QT, CH):
            w = min(CH, QT - jc)
            qt_ld = ld.tile([DIM, CH * 128], bf16, tag="kt")
            nc.sync.dma_start(qt_ld[:, : w * 128], qT[:, jc * 128:(jc + w) * 128])
            q_sb8 = kvp.tile([128, CH * DIM], f32, tag="qout")
            for j in range(w):
                acc = ps.tile([128, DIM], f32, space="PSUM", tag="mm")
                nc.tensor.matmul(out=acc[:], lhsT=qt_ld[:, j * 128:(j + 1) * 128],
                                 rhs=wq_sb[:], start=True, stop=True)
                nc.vector.tensor_tensor(
                    out=q_sb8[:, j * DIM:(j + 1) * DIM], in0=acc[:],
                    in1=bq_sb[:], op=mybir.AluOpType.add)
            nc.sync.dma_start(
                bass.AP(q_tab.ap()[:, :].tensor, jc * 128 * DIM,
                        [[DIM, 128], [128 * DIM, w], [1, DIM]]),
                bass.AP(q_sb8[:].tensor, q_sb8[:].offset,
                        [list(q_sb8[:].ap[0]), [DIM, w], [1, DIM]]))

        # ---- G: main edge loop ----
        g_st = int(os.environ.get("KERNEL_MAX_ST", str(ST)))
        for s in range(min(ST, g_st)):
            cc = st_region[s]
            tile0 = s * TC - meta["REG_TILE0"][cc]  # local tile idx of this st
            ikv = smal.tile([128, SCOLS * 8], i16, tag="ikv")
            nc.sync.dma_start(ikv[:], kv_idx16[:, s * SCOLS * 8:(s + 1) * SCOLS * 8])
            iq = smal.tile([128, TC * 8], i16, tag="iq")
            nc.sync.dma_start(iq[:], q_idx16[:, s * TC * 8:(s + 1) * TC * 8])
            mk = smal.tile([128, SCOLS * H], f32, tag="mk")
            nc.scalar.dma_start(mk[:], mskap[:, s * SCOLS * H:(s + 1) * SCOLS * H])
            crw = smal.tile([128, TC], i16, tag="crw")
            nc.scalar.dma_start(crw[:], crw16[:, s * TC:(s + 1) * TC])

            kv_g = gat.tile([128, SCOLS, 128], f32, tag="kv_g")
            nc.gpsimd.dma_gather(kv_g[:, :, :], kv_tab.ap()[cc * CHUNK:, :],
                                 ikv[:], SCOLS * 128, SCOLS * 128, 128,
                                 single_packet=False)
            q_g = smal.tile([128, TC, DIM], f32, tag="q_g")
            nc.gpsimd.dma_gather(q_g[:, :, :], q_tab.ap()[:, :], iq[:],
                                 TC * 128, TC * 128, DIM)

            # prod[p, (t,j), f] = kv_g[p, t*4+j, f] * q_g[p, t, f]
            prod = work.tile([128, SCOLS, DIM], f32, tag="prod")
            for j in range(D_PAD):
                nc.vector.tensor_tensor(
                    out=ap4(prod, [[D_PAD * DIM, TC], [1, DIM]], extra_off=j * DIM),
                    in0=ap4(kv_g, [[D_PAD * 128, TC], [1, DIM]], extra_off=j * 128),
                    in1=ap4(q_g, [[DIM, TC], [1, DIM]]),
                    op=mybir.AluOpType.mult)
            # sco[p, (t,j), h] = sum_dk prod
            sco = smal.tile([128, SCOLS, H], f32, tag="sco")
            for h in range(H):
                nc.vector.tensor_reduce(
                    out=ap4(sco, [[H, SCOLS], [1, 1]], extra_off=h),
                    in_=ap4(prod, [[DIM, SCOLS], [1, DK]], extra_off=h * DK),
                    axis=mybir.AxisListType.X, op=mybir.AluOpType.add,
                    opt_input=False, opt_output=False)
            nc.vector.tensor_tensor(
                out=ap4(sco, [[1, SCOLS * H]]), in0=ap4(sco, [[1, SCOLS * H]]),
                in1=mk[:], op=mybir.AluOpType.add)
            ex = smal.tile([128, SCOLS, H], f32, tag="ex")
            nc.scalar.activation(out=ex[:], in_=sco[:],
                                 func=mybir.ActivationFunctionType.Exp,
                                 scale=1.0 / np.sqrt(DK))
            exe = work.tile([128, SCOLS, DIM], f32, tag="exe")
            nc.scalar.activation(out=exe[:],
                                 in_=ap4(sco, [[1, SCOLS * H], [0, DK]]),
                                 func=mybir.ActivationFunctionType.Exp,
                                 scale=1.0 / np.sqrt(DK))
            wv = prod
            for j in range(D_PAD):
                nc.vector.tensor_tensor(
                    out=ap4(wv, [[D_PAD * DIM, TC], [1, DIM]], extra_off=j * DIM),
                    in0=ap4(kv_g, [[D_PAD * 128, TC], [1, DIM]],
                            extra_off=j * 128 + DIM),
                    in1=ap4(exe, [[D_PAD * DIM, TC], [1, DIM]], extra_off=j * DIM),
                    op=mybir.AluOpType.mult)
            # ad[p, t, 0:64] = sum_j wv ; ad[p, t, 64:68] = sum_j ex
            ad = smal.tile([128, TC, 68], f32, tag="ad")
            for j in range(D_PAD):
                if j == 0:
                    nc.vector.tensor_copy(
                        out=ap4(ad, [[68, TC], [1, DIM]]),
                        in_=ap4(wv, [[D_PAD * DIM, TC], [1, DIM]]))
                    nc.vector.tensor_copy(
                        out=ap4(ad, [[68, TC], [1, H]], extra_off=DIM),
                        in_=ap4(ex, [[D_PAD * H, TC], [1, H]]))
                else:
                    nc.vector.tensor_tensor(
                        out=ap4(ad, [[68, TC], [1, DIM]]),
                        in0=ap4(ad, [[68, TC], [1, DIM]]),
                        in1=ap4(wv, [[D_PAD * DIM, TC], [1, DIM]],
                                extra_off=j * DIM),
                        op=mybir.AluOpType.add)
                    nc.vector.tensor_tensor(
                        out=ap4(ad, [[68, TC], [1, H]], extra_off=DIM),
                        in0=ap4(ad, [[68, TC], [1, H]], extra_off=DIM),
                        in1=ap4(ex, [[D_PAD * H, TC], [1, H]], extra_off=j * H),
                        op=mybir.AluOpType.add)
            if s == 0 and dbg:
                nc.sync.dma_start(dbg["dbg_kvg"][:, :],
                                  ap4(kv_g, [[1, SCOLS * 128]]))
                nc.sync.dma_start(dbg["dbg_sco"][:, :], ap4(sco, [[1, SCOLS * H]]))
                nc.sync.dma_start(dbg["dbg_ad"][:, :], ap4(ad, [[1, TC * 68]]))
                nc.sync.dma_start(dbg["dbg_qg"][:, :], ap4(q_g, [[1, TC * DIM]]))
            # combine rows -> node columns; csb[p=col, t, 0:68]
            csb = outp.tile([128, TC, 128], bf16, tag="csb")
            crf = smal.tile([128, TC], f32, tag="crf")
            nc.vector.tensor_copy(crf[:], crw[:])
            for t in range(TC):
                oh = work.tile([128, 128], f32, tag="oh")
                nc.vector.tensor_tensor(
                    out=oh[:], in0=iota_f[:],
                    in1=crf[:, t:t + 1].to_broadcast([128, 128]),
                    op=mybir.AluOpType.is_equal)
                cps = ps.tile([128, 68], f32, space="PSUM", tag="x")
                nc.tensor.matmul(out=cps[:], lhsT=oh[:], rhs=ad[:, t, :],
                                 start=True, stop=True)
                nc.vector.tensor_copy(csb[:, t, 0:68], cps[:])
            # write supertile's tiles to comb2 region block
            nc.scalar.dma_start(
                bass.AP(comb2.ap()[:, :].tensor,
                        (REG_ROW0[cc] + tile0) * 128,
                        [[T_C[cc] * 128, 128], [128, TC], [1, 128]]),
                bass.AP(csb[:].tensor, csb[:].offset,
                        [list(csb[:].ap[0]), [128, TC], [1, 128]]))

        # ---- F: gather partials, normalize, project ----
        n_slab = 0 if os.environ.get("KERNEL_SKIP_F") else NSLAB
        for sl in range(n_slab):
            fsum = work.tile([128, FCOLS, 128], f32, tag="fsum")
            for cc in range(NCHUNK):
                fg = gat.tile([128, FCOLS, 128], bf16, tag="fg")
                fi = smal.tile([128, SLAB // 16], i16, tag="fi")
                nc.sync.dma_start(
                    fi[:], fidx16[:, (cc * NODES_PAD + sl * SLAB) // 16:
                                  (cc * NODES_PAD + (sl + 1) * SLAB) // 16])
                nc.gpsimd.dma_gather(fg[:, :, :],
                                     comb2.ap()[REG_ROW0[cc]:, :], fi[:],
                                     SLAB, SLAB, 128, single_packet=False)
                if cc == 0:
                    nc.vector.tensor_copy(ap4(fsum, [[1, FCOLS * 128]]),
                                          ap4(fg, [[1, FCOLS * 128]]))
                else:
                    nc.vector.tensor_tensor(
                        out=ap4(fsum, [[1, FCOLS * 128]]),
                        in0=ap4(fsum, [[1, FCOLS * 128]]),
                        in1=ap4(fg, [[1, FCOLS * 128]]),
                        op=mybir.AluOpType.add)
            osb = outp.tile([128, FCOLS, DIM], f32, tag="osb")
            for f in range(FCOLS):
                dn = smal.tile([128, H], f32, tag="dn")
                nc.vector.tensor_scalar(
                    out=dn[:], in0=fsum[:, f, DIM:DIM + H], scalar1=1e-30,
                    scalar2=None, op0=mybir.AluOpType.max)
                rd = smal.tile([128, H], f32, tag="rd")
                nc.vector.reciprocal(rd[:], dn[:])
                nrm = outp.tile([128, DIM], f32, tag="nrm")
                nc.vector.tensor_tensor(
                    out=nrm[:], in0=fsum[:, f, 0:DIM],
                    in1=ap4(rd, [[1, H], [0, DK]]),
                    op=mybir.AluOpType.mult)
                tps = ps.tile([DIM, 128], f32, space="PSUM", tag="x")
                nc.tensor.transpose(out=tps[:], in_=nrm[:], identity=ident[:])
                nrmT = outp.tile([DIM, 128], f32, tag="nrmT")
                nc.vector.tensor_copy(nrmT[:], tps[:])
                ops_ = ps.tile([128, DIM], f32, space="PSUM", tag="mm")
                nc.tensor.matmul(out=ops_[:], lhsT=nrmT[:], rhs=wo_sb[:],
                                 start=True, stop=True)
                nc.vector.tensor_tensor(
                    out=osb[:, f, :], in0=ops_[:], in1=bo_sb[:],
                    op=mybir.AluOpType.add)
            nc.sync.dma_start(
                bass.AP(out[:, :].tensor, sl * SLAB * DIM,
                        [[DIM, 128], [128 * DIM, FCOLS], [1, DIM]]),
                bass.AP(osb[:].tensor, osb[:].offset,
                        [list(osb[:].ap[0]), [DIM, FCOLS], [1, DIM]]))

    nc.compile()
    return nc


def kernel(**inputs):
    from concourse.bass_utils import run_bass_kernel_spmd

    query = np.asarray(inputs["query"], np.float32)
    key = np.asarray(inputs["key"], np.float32)
    value = np.asarray(inputs["value"], np.float32)
    src = np.asarray(inputs["src"])
    dst = np.asarray(inputs["dst"])
    Wq = np.asarray(inputs["Wq"], np.float32)
    bq = np.asarray(inputs["bq"], np.float32)
    Wk = np.asarray(inputs["Wk"], np.float32)
    bk = np.asarray(inputs["bk"], np.float32)
    Wv = np.asarray(inputs["Wv"], np.float32)
    bv = np.asarray(inputs["bv"], np.float32)
    Wo = np.asarray(inputs["Wo"], np.float32)
    bo = np.asarray(inputs["bo"], np.float32)

    cores, meta = _host_prep(src, dst)
    nc = _build_program(meta)

    import ml_dtypes
    bf = ml_dtypes.bfloat16
    keyT = np.zeros((DIM, KV_PAD), bf)
    keyT[:, :N] = key.T.astype(bf)
    valT = np.zeros((DIM, KV_PAD), bf)
    valT[:, :N] = value.T.astype(bf)
    wkv = np.zeros((128, 128), np.float32)
    wkv[0:DIM, 0:DIM] = Wk.T
    wkv[DIM:128, DIM:128] = Wv.T
    bkv = np.broadcast_to(np.concatenate([bk, bv]), (128, 128)).astype(np.float32).copy()
    ident = np.eye(128, dtype=np.float32)
    iota = np.broadcast_to(np.arange(128, dtype=np.float32), (128, 128)).copy()

    in_maps = []
    for c in cores:
        qT = np.zeros((DIM, NODES_PAD), bf)
        qT[:, : c["nn"]] = query[c["n0"]:c["n1"]].T.astype(bf)
        in_maps.append(dict(
            keyT=keyT, valT=valT, qT=qT, wkv=wkv.astype(bf), bkv=bkv,
            wqT=Wq.T.astype(bf).copy(),
            bq=np.broadcast_to(bq, (128, DIM)).astype(np.float32).copy(),
            woT=Wo.T.copy().astype(np.float32),
            bo=np.broadcast_to(bo, (128, DIM)).astype(np.float32).copy(),
            identT=ident, iotaT=iota,
            kv_idx16=c["kv_idx16"], q_idx16=c["q_idx16"], msk=c["msk"],
            crw16=c["crw16"], fidx16=c["fidx16"]))

    trace = bool(int(os.environ.get("KERNEL_TRACE", "0")))
    res = run_bass_kernel_spmd(
        nc, in_maps, core_ids=list(range(NCORES)), trace=trace,
        tmpdir=os.environ.get("KERNEL_TRACE_DIR") or None)
    kernel.last_results = res

    out = np.empty((N, DIM), np.float32)
    for c, r in zip(cores, res.results):
        out[c["n0"]:c["n1"]] = r["out"][: c["nn"]]
    return out
